# revision 2
# baseline (speedup 1.0000x reference)
"""Viterbi decode (CRF layer) on Trainium2 — Bass kernel.

Problem: feats [1024, 512, 128] f32, transitions [128, 128],
start/stop_transitions [128] -> best tag sequence [1024, 512] int32.

Strategy: pure batch data-parallelism across 8 NeuronCores. Each core takes
128 batch rows (= 128 SBUF partitions) and runs the sequential max-plus
forward scan on-chip:

    sc[b, i, j] = v[b, i] + trans[i, j]          (fp32, one rounding)
    mx[b, j]    = max_i sc[b, i, j]
    v'[b, j]    = mx[b, j] + feats[b, t, j]      (fp32, one rounding)

The per-step state vectors v stream to DRAM; the backtrace recomputes the
argmax only along the traced path (B*S tiny argmaxes) on host during the
unshard step, with identical fp32 arithmetic and first-index tie-breaking,
so the final int32 tags match the reference bit-exactly.

variant="v0" keeps the full device-side backpointer computation (slower,
fully self-contained backpointers) as a fallback.
"""

import numpy as np

B, S, T = 1024, 512, 128
NCORES = 8
BL = B // NCORES  # 128 batch rows per core == SBUF partition count


def build_viterbi_nc(trans_np, S_=S, T_=T, BL_=BL, variant="v1"):
    """Build the per-core Bass program (same NEFF for all cores).

    NOTE: start_transitions must already be folded into feats[:, 0, :] by the
    caller (bit-exact: same single fp32 add the reference performs).

    walrus/core_v3 allows only ONE attached sync-wait per compute
    instruction; the initial state goes through a DVE tensor_copy so every
    instruction waits on at most one foreign semaphore.
    """
    import concourse.bacc as bacc
    import concourse.mybir as mybir
    import concourse.tile as tile

    f32 = mybir.dt.float32
    add = mybir.AluOpType.add
    mx_op = mybir.AluOpType.max
    eq_op = mybir.AluOpType.is_equal
    mul_op = mybir.AluOpType.mult
    X = mybir.AxisListType.X

    nc = bacc.Bacc("TRN2", target_bir_lowering=False, debug=False)
    feats = nc.declare_dram_parameter("feats", [BL_, S_, T_], f32, isOutput=False)
    if variant == "v0":
        bp = nc.declare_dram_parameter("bp", [S_ - 1, BL_, T_], f32, isOutput=True)
    else:
        vs_out = nc.declare_dram_parameter("vs", [S_ - 1, BL_, T_], f32, isOutput=True)
    v_final = nc.declare_dram_parameter("v_final", [BL_, T_], f32, isOutput=True)

    if variant == "v2":
        # table stored [j, i] (transposed) so the score buffer is written and
        # reduced fully contiguously in [b, j, i] order
        tbl = np.ascontiguousarray(trans_np.T.reshape(1, T_ * T_), dtype=np.float32)
    else:
        tbl = np.ascontiguousarray(trans_np.reshape(1, T_ * T_), dtype=np.float32)
    tbc_d = nc.inline_tensor(tbl, "tbc")
    iota_d = nc.inline_tensor(
        np.arange(T_ - 1, -1, -1, dtype=np.float32).reshape(1, T_), "iotad"
    )

    with tile.TileContext(nc) as tc:
        with (
            tc.tile_pool(name="const", bufs=1) as cpool,
            tc.tile_pool(name="feat", bufs=8) as fpool,
            tc.tile_pool(name="vst", bufs=4) as vpool,
            tc.tile_pool(name="sc", bufs=1 if variant == "v0" else 2) as scpool,
            tc.tile_pool(name="mx", bufs=2) as mxpool,
            tc.tile_pool(name="bpp", bufs=4) as bppool,
        ):
            tbc = cpool.tile([BL_, T_ * T_], f32, tag="tbc")
            nc.gpsimd.dma_start(tbc[:, :], tbc_d[:, :].partition_broadcast(BL_))
            iotab = cpool.tile([BL_, T_], f32, tag="iotab")
            nc.gpsimd.dma_start(iotab[:, :], iota_d[:, :].partition_broadcast(BL_))

            f0 = fpool.tile([BL_, T_], f32, tag="feat")
            nc.gpsimd.dma_start(f0[:, :], feats[:, 0, :])
            v = vpool.tile([BL_, T_], f32, tag="v")
            nc.vector.tensor_copy(v[:, :], f0[:, :])

            tb3 = tbc[:, :].rearrange("p (i j) -> p i j", i=T_)
            io3 = iotab[:, :].unsqueeze(-1).broadcast_to([BL_, T_, T_])
            # v2: table is [j, i]-major; split the add by j between DVE and
            # Pool (Pool ~2x slower -> give it the smaller range)
            import os as _os
            JSPLIT = int(_os.environ.get("VT_JSPLIT", T_))
            DSPLIT = int(_os.environ.get("VT_DSPLIT", T_ // 2))

            for t in range(1, S_):
                ft = fpool.tile([BL_, T_], f32, tag="feat")
                nc.gpsimd.dma_start(ft[:, :], feats[:, t, :])

                sc = scpool.tile([BL_, T_ * T_], f32, tag="sc")
                sc3 = sc[:, :].rearrange("p (i j) -> p i j", i=T_)
                scT = sc[:, :].rearrange("p (i j) -> p j i", i=T_)
                mxt = mxpool.tile([BL_, T_], f32, tag="mx")

                if variant == "v2":
                    # sc[b, j, i] = v[b, i] + tT[j, i]; contiguous writes
                    scJ = sc[:, :].rearrange("p (j i) -> p j i", j=T_)
                    tbJ = tbc[:, :].rearrange("p (j i) -> p j i", j=T_)
                    nA = JSPLIT * T_
                    v3a = v[:, :].unsqueeze(1).broadcast_to([BL_, JSPLIT, T_])
                    scA = sc[:, 0:nA].rearrange("p (j i) -> p j i", j=JSPLIT)
                    tbA = tbc[:, 0:nA].rearrange("p (j i) -> p j i", j=JSPLIT)
                    nc.vector.tensor_tensor(scA, v3a, tbA, add)
                    if JSPLIT < T_:
                        v3b = v[:, :].unsqueeze(1).broadcast_to(
                            [BL_, T_ - JSPLIT, T_]
                        )
                        scB = sc[:, nA : T_ * T_].rearrange(
                            "p (j i) -> p j i", j=T_ - JSPLIT
                        )
                        tbB = tbc[:, nA : T_ * T_].rearrange(
                            "p (j i) -> p j i", j=T_ - JSPLIT
                        )
                        nc.gpsimd.tensor_tensor(scB, v3b, tbB, add)
                    nc.vector.tensor_reduce(mxt[:, :], scJ, axis=X, op=mx_op)
                elif variant == "v3":
                    # sc[b,i,j] = t[i,j] + v[b,i]: DVE does rows [0, DSPLIT)
                    # in one tensor_tensor; ACT does rows [DSPLIT, T) as
                    # per-row activation-adds (bias = per-partition scalar)
                    nD = DSPLIT * T_
                    v3a = v[:, 0:DSPLIT].unsqueeze(-1).broadcast_to(
                        [BL_, DSPLIT, T_]
                    )
                    scA = sc[:, 0:nD].rearrange("p (i j) -> p i j", i=DSPLIT)
                    tbA = tbc[:, 0:nD].rearrange("p (i j) -> p i j", i=DSPLIT)
                    nc.vector.tensor_tensor(scA, v3a, tbA, add)
                    for i in range(DSPLIT, T_):
                        nc.scalar.add(
                            sc[:, i * T_ : (i + 1) * T_],
                            tbc[:, i * T_ : (i + 1) * T_],
                            v[:, i : i + 1],
                        )
                    nc.vector.tensor_reduce(mxt[:, :], scT, axis=X, op=mx_op)
                else:
                    v3 = v[:, :].unsqueeze(-1).broadcast_to([BL_, T_, T_])
                    nc.vector.tensor_tensor(sc3, v3, tb3, add)
                    nc.vector.tensor_reduce(mxt[:, :], scT, axis=X, op=mx_op)

                vn = vpool.tile([BL_, T_], f32, tag="v")
                nc.vector.tensor_tensor(vn[:, :], mxt[:, :], ft[:, :], add)

                if variant == "v0":
                    # backpointers on device: sc <- (sc==mx)*(T-1-i); bp=max_i
                    mx3 = mxt[:, :].unsqueeze(1).broadcast_to([BL_, T_, T_])
                    nc.vector.tensor_tensor(sc3, sc3, mx3, eq_op)
                    nc.vector.tensor_tensor(sc3, sc3, io3, mul_op)
                    bpt = bppool.tile([BL_, T_], f32, tag="bp")
                    nc.vector.tensor_reduce(bpt[:, :], scT, axis=X, op=mx_op)
                    nc.gpsimd.dma_start(bp[t - 1, :, :], bpt[:, :])
                else:
                    nc.gpsimd.dma_start(vs_out[t - 1, :, :], vn[:, :])

                v = vn

            nc.gpsimd.dma_start(v_final[:, :], v[:, :])
    nc.finalize()
    return nc


def _run(nc, in_maps, **kwargs):
    from concourse.bass_utils import run_bass_kernel_spmd

    return run_bass_kernel_spmd(nc, in_maps, core_ids=list(range(len(in_maps))), **kwargs)


def _backtrace_from_vs(vs, v0, trans, stop):
    """Exact backtrace from per-step state vectors.

    vs: [B, S-1, T] fp32 (v at t=1..S-1), v0: [B, T] (v at t=0).
    Recomputes argmax_i(v[t-1,:,i] + trans[i, j_t]) along the traced path
    only — identical fp32 arithmetic + first-index ties as the reference.
    """
    B_, Sm1, T_ = vs.shape
    S_ = Sm1 + 1
    last = np.argmax(vs[:, -1, :] + stop[None, :], axis=1).astype(np.int32)
    tags = np.empty((B_, S_), dtype=np.int32)
    tags[:, -1] = last
    cur = last
    transT = np.ascontiguousarray(trans.T)  # [j, i]
    for t in range(S_ - 1, 0, -1):
        vprev = vs[:, t - 2, :] if t >= 2 else v0
        col = vprev + transT[cur]  # [B, T] fp32: v[b,t-1,i] + trans[i, j_t]
        cur = np.argmax(col, axis=1).astype(np.int32)
        tags[:, t - 1] = cur
    return tags


def kernel(feats, transitions, start_transitions, stop_transitions, _trace=False,
           _variant="v2"):
    feats = np.asarray(feats, dtype=np.float32).copy()
    trans = np.ascontiguousarray(np.asarray(transitions, dtype=np.float32))
    start = np.ascontiguousarray(np.asarray(start_transitions, dtype=np.float32))
    stop = np.ascontiguousarray(np.asarray(stop_transitions, dtype=np.float32))
    assert feats.shape == (B, S, T)

    feats[:, 0, :] += start  # fold start_transitions (bit-exact vs reference)

    nc = build_viterbi_nc(trans, variant=_variant)
    in_maps = [{"feats": feats[c * BL : (c + 1) * BL]} for c in range(NCORES)]
    res = _run(nc, in_maps, trace=_trace)

    if _variant == "v0":
        bp_f = np.concatenate(
            [np.transpose(r["bp"], (1, 0, 2)) for r in res.results], axis=0
        )
        v_fin = np.concatenate([r["v_final"] for r in res.results], axis=0)
        idx = (T - 1) - bp_f.astype(np.int32)
        last = np.argmax(v_fin + stop[None, :], axis=1).astype(np.int32)
        tags = np.empty((B, S), dtype=np.int32)
        tags[:, S - 1] = last
        cur = last
        ar = np.arange(B)
        for t in range(S - 2, -1, -1):
            cur = idx[ar, t, cur]
            tags[:, t] = cur
    else:
        vs = np.concatenate(
            [np.transpose(r["vs"], (1, 0, 2)) for r in res.results], axis=0
        )  # [B, S-1, T]
        v0 = feats[:, 0, :]  # start already folded
        tags = _backtrace_from_vs(vs, v0, trans, stop)

    if _trace:
        return tags, res
    return tags

